# revision 21
# baseline (speedup 1.0000x reference)
"""Trainium2 Bass kernel for nn_CustomAttentionLayer (GQA attention + RoPE + o_proj).

v5: residual-fp8 DoubleRow projections + per-group earliest collective staging.

Sharding: 8-way over (batch, query-chunk): core c handles batch c//4, query
rows [(c%4)*512, (c%4)*512+512). Each core computes k/v projections for its
own 512 tokens, all-gathers (k|v) per kv-head-group across the 4 cores of its
batch, then runs attention + o_proj for its 512 query rows.

Precision scheme (validated vs reference, metric ~7e-3 < 2e-2 gate):
- All four projections (q/k/v/o) run as THREE fp8(e4m3) DoubleRow matmul
  streams accumulating in ONE f32 PSUM group:
      x@W ~ x8@W8 + x8@Wr + xr@W8       (xr = x*s - x8 residual, same scale)
  Host pre-scales h and the weights by per-tensor pow2 factors into e4m3
  range; residuals share the scale so no separate PSUM group is needed.
  DoubleRow processes 2 contraction k-tiles per instruction at 0.5
  cycles/row: 24 DR matmuls replace 16 bf16 matmuls per 2048-contraction
  unit (75% of the bf16 PE cost).
- Scores (q.k), exp, pv, rowsum stay bf16 (fp8 there fails the 2e-2 gate;
  measured: any single fp8 stage without residual is 2.4e-2..6.6e-2).
- Projection scales are folded downstream for free: q/k into the rope
  cos/sin tables, v into the softmax-normalize scalar, o into the final
  PSUM evacuation scalar. ctx (attention output) is quantized on-device to
  fp8+residual (s_c=128) for the o_proj DoubleRow streams.

Engine budget (TimelineSim): PE 215us (kv 20.5 + q 41 + scores 54.6 +
pv 54.6 + rowsum 3.4 + o 41), Act ~150 (exp 133 + q/k/v evac), DVE ~165
(tree 82 + rope 39 + recip + norm STTs + o evac), Pool ~13 (ctx residual).
Collectives 4 x 41us serialized, staged per-group at the earliest moment.

Queue discipline:
- Pool queue: staging DMAs, collectives, per-group gather imports (emitted
  at the consuming group's head), and the ctx-residual STTs. Keeps all
  collective-dependent waits off the SP DMA stream.
- SP queue: weight/h streams, wq per-head prefetch, wo per-n prefetch, out.
"""

import os
import numpy as np
import ml_dtypes

import concourse.bass as bass
import concourse.mybir as mybir
import concourse.tile as tile
from concourse import bacc
from concourse.bass_utils import run_bass_kernel_spmd

B, S, H = 2, 2048, 2048
NH, NKV, HD = 16, 4, 128
SQ = 512                      # query rows per core
NC = 8                        # cores
KT = H // 128                 # 16 contraction tiles over H
SJ = S // 128                 # 16 key-position tiles
NP = KT // 2                  # 8 DoubleRow contraction pairs
SCALE = 1.0 / float(np.sqrt(HD))
GRPS = [[0, 1, 2, 3], [4, 5, 6, 7]]
CHUNK = 128 * 1024            # elems of one core's (k_g | v_g) staging
S_C = 128.0                   # ctx fp8 scale (ctx absmax ~0.57)

f32 = mybir.dt.float32
bf16 = mybir.dt.bfloat16
f8 = mybir.dt.float8e4
FP = mybir.ActivationFunctionType
ALU = mybir.AluOpType
DR = mybir.MatmulPerfMode.DoubleRow
BF16 = ml_dtypes.bfloat16
F8NP = ml_dtypes.float8_e4m3


def _body(nc, tc, t, norm_sc, o_sc):
    h8P, hrP = t["h8P"], t["hrP"]
    wq8P, wqrP = t["wq8P"], t["wqrP"]
    wk8P, wkrP = t["wk8P"], t["wkrP"]
    wv8P, wvrP = t["wv8P"], t["wvrP"]
    wo8P, worP = t["wo8P"], t["worP"]
    ccssP, outD = t["ccss"], t["out"]
    stgs = [t[f"stg{g}"] for g in range(NKV)]
    gths = [t[f"gth{g}"] for g in range(NKV)]

    with tc.tile_pool(name="main", bufs=1) as mp, tc.tile_pool(
        name="proj", bufs=1
    ) as pt, tc.tile_pool(name="attn", bufs=1) as at, tc.tile_pool(
        name="oproj", bufs=1
    ) as ot, tc.tile_pool(name="psum", bufs=1, space="PSUM") as pp:
        # ---- persistent SBUF state -------------------------------------
        qts = [mp.tile([128, SQ], bf16, tag="qt", bufs=NH, name=f"qt{h}")
               for h in range(NH)]                      # q rope'd
        kts = [mp.tile([128, S], bf16, tag="kt", bufs=NKV, name=f"kt{g}")
               for g in range(NKV)]                     # k gathered [hd, keys]
        vimp = [mp.tile([128, SJ * 128], bf16, tag="vi", bufs=NKV,
                        name=f"vi{g}") for g in range(NKV)]  # v gathered
        kown = mp.tile([128, NKV * 512], bf16, tag="kown", bufs=1)
        vown = mp.tile([128, NKV * 512], bf16, tag="vown", bufs=1)
        ctx8 = mp.tile([128, NH * 512], f8, tag="ctx8", bufs=1)
        ctxr = mp.tile([128, NH * 512], f8, tag="ctxr", bufs=1)
        ones = mp.tile([128, 128], bf16, tag="ones", bufs=1)
        nc.vector.memset(ones[:], 1.0)
        ones512 = mp.tile([128, 512], bf16, tag="ones512", bufs=1)
        nc.vector.memset(ones512[:], 1.0)

        # ---- input streams (SP queue) ----------------------------------
        h8t = pt.tile([128, KT * 512], f8, tag="h8", bufs=1)
        hrt = pt.tile([128, KT * 512], f8, tag="hr", bufs=1)
        wk8 = pt.tile([128, NKV * KT * 128], f8, tag="wk8", bufs=1)
        wkr = pt.tile([128, NKV * KT * 128], f8, tag="wkr", bufs=1)
        wv8 = pt.tile([128, NKV * KT * 128], f8, tag="wv8", bufs=1)
        wvr = pt.tile([128, NKV * KT * 128], f8, tag="wvr", bufs=1)
        # kproj g0 needs all h8 + wk8/wkr g0-slice first, then hr
        for c4 in range(4):
            nc.sync.dma_start(h8t[:, bass.ts(c4, KT * 128)],
                              h8P[:, bass.ts(c4, KT * 128)])
        nc.sync.dma_start(wk8[:, bass.ts(0, KT * 128)],
                          wk8P[:, bass.ts(0, KT * 128)])
        nc.sync.dma_start(wkr[:, bass.ts(0, KT * 128)],
                          wkrP[:, bass.ts(0, KT * 128)])
        for c4 in range(4):
            nc.sync.dma_start(hrt[:, bass.ts(c4, KT * 128)],
                              hrP[:, bass.ts(c4, KT * 128)])
        nc.sync.dma_start(wv8[:, bass.ts(0, KT * 128)],
                          wv8P[:, bass.ts(0, KT * 128)])
        nc.sync.dma_start(wvr[:, bass.ts(0, KT * 128)],
                          wvrP[:, bass.ts(0, KT * 128)])
        for g in range(1, NKV):
            nc.sync.dma_start(wk8[:, bass.ts(g, KT * 128)],
                              wk8P[:, bass.ts(g, KT * 128)])
            nc.sync.dma_start(wkr[:, bass.ts(g, KT * 128)],
                              wkrP[:, bass.ts(g, KT * 128)])
            nc.sync.dma_start(wv8[:, bass.ts(g, KT * 128)],
                              wv8P[:, bass.ts(g, KT * 128)])
            nc.sync.dma_start(wvr[:, bass.ts(g, KT * 128)],
                              wvrP[:, bass.ts(g, KT * 128)])
        ccss = pt.tile([128, 1024], bf16, tag="ccss", bufs=1)
        nc.sync.dma_start(ccss[:], ccssP[:, :])
        ccq = ccss[:, 0:512]
        ssq = ccss[:, 512:1024]

        h83 = h8t[:].rearrange("p (k j) -> p k j", j=512)
        hr3 = hrt[:].rearrange("p (k j) -> p k j", j=512)

        wqm_tiles = {}

        def prefetch_wq(m):
            if m < NH:
                wq8m = pt.tile([128, KT * 128], f8, tag="wq8m", bufs=4,
                               name="wq8m")
                wqrm = pt.tile([128, KT * 128], f8, tag="wqrm", bufs=4,
                               name="wqrm")
                nc.sync.dma_start(wq8m[:], wq8P[:, bass.ts(m, KT * 128)])
                nc.sync.dma_start(wqrm[:], wqrP[:, bass.ts(m, KT * 128)])
                wqm_tiles[m] = (wq8m, wqrm)

        for _m in range(4):
            prefetch_wq(_m)

        def rope(dst_r, dst_i, x):
            # x = [xr; xi] on partition halves; cc/ss = [c; c], [s; s].
            # Both INPUTS of a TensorTensor must share the start partition.
            t1 = at.tile([64, 512], bf16, tag="ropeA", bufs=2, name="t1")
            t2 = at.tile([64, 512], bf16, tag="ropeB", bufs=2, name="t2")
            t3 = at.tile([64, 512], bf16, tag="ropeC", bufs=2, name="t3")
            t4 = at.tile([64, 512], bf16, tag="ropeD", bufs=2, name="t4")
            nc.vector.tensor_tensor(t1[:], x[64:128, :], ssq[64:128, :],
                                    op=ALU.mult)
            nc.vector.tensor_tensor(t2[:], x[0:64, :], ccq[0:64, :],
                                    op=ALU.mult)
            nc.vector.tensor_sub(dst_r, t2[:], t1[:])
            nc.vector.tensor_tensor(t3[:], x[0:64, :], ssq[0:64, :],
                                    op=ALU.mult)
            nc.vector.tensor_tensor(t4[:], x[64:128, :], ccq[64:128, :],
                                    op=ALU.mult)
            nc.vector.tensor_add(dst_i, t3[:], t4[:])

        def w_pair(wt, g, tp):
            return wt[:, g * KT * 128 + 2 * tp * 128:
                      g * KT * 128 + (2 * tp + 2) * 128].rearrange(
                          "p (two m) -> p two m", two=2)

        def h_pair(ht3, tp):
            return ht3[:, 2 * tp:2 * tp + 2, :]

        def kproj(g):
            psk = pp.tile([128, 512], f32, tag="mm", bufs=2, name="psk")
            idx = 0
            for wt, ht3 in ((wk8, h83), (wkr, h83), (wk8, hr3)):
                for tp in range(NP):
                    nc.tensor.matmul(
                        psk[:], w_pair(wt, g, tp), h_pair(ht3, tp),
                        start=(idx == 0), stop=(idx == 3 * NP - 1),
                        perf_mode=DR,
                    )
                    idx += 1
            sk = at.tile([128, 512], bf16, tag="sqk", bufs=3, name="sk")
            nc.scalar.copy(sk[:], psk[:])
            rope(kown[0:64, g * 512:(g + 1) * 512],
                 kown[64:128, g * 512:(g + 1) * 512], sk)

        def vproj(g):
            # psv [128 tok, 128 hd] per token block; scales fold into norm.
            for tok in range(4):
                pvt = pp.tile([128, 512], f32, tag="mm", bufs=2, name="psv")
                pv = pvt[:, 0:128]
                idx = 0
                for ht3, wt in ((h83, wv8), (h83, wvr), (hr3, wv8)):
                    for tp in range(NP):
                        nc.tensor.matmul(
                            pv,
                            ht3[:, 2 * tp:2 * tp + 2,
                                tok * 128:(tok + 1) * 128],
                            w_pair(wt, g, tp),
                            start=(idx == 0), stop=(idx == 3 * NP - 1),
                            perf_mode=DR,
                        )
                        idx += 1
                nc.scalar.copy(
                    vown[:, g * 512 + tok * 128: g * 512 + (tok + 1) * 128],
                    pv)

        def stage_cc(g):
            # Pool queue: staging DMAs + collective; keeps waits off SP.
            stgv = stgs[g].rearrange("1 (p j) -> p j", p=128)
            nc.gpsimd.dma_start(stgv[:, 0:512], kown[:, bass.ts(g, 512)])
            nc.gpsimd.dma_start(stgv[:, 512:1024], vown[:, bass.ts(g, 512)])
            nc.gpsimd.collective_compute(
                "AllGather", ALU.bypass, replica_groups=GRPS,
                ins=[stgs[g]], outs=[gths[g]],
            )

        def qproj(m):
            wq8m, wqrm = wqm_tiles.pop(m)
            prefetch_wq(m + 4)
            psq = pp.tile([128, 512], f32, tag="mm", bufs=2, name="psq")
            idx = 0
            for wt, ht3 in ((wq8m, h83), (wqrm, h83), (wq8m, hr3)):
                for tp in range(NP):
                    nc.tensor.matmul(
                        psq[:], wt[:, bass.ts(tp, 256)].rearrange(
                            "p (two m) -> p two m", two=2),
                        h_pair(ht3, tp),
                        start=(idx == 0), stop=(idx == 3 * NP - 1),
                        perf_mode=DR,
                    )
                    idx += 1
            sq = at.tile([128, 512], bf16, tag="sqk", bufs=3, name="sq")
            nc.scalar.copy(sq[:], psq[:])
            rope(qts[m][0:64, :], qts[m][64:128, :], sq)

        # ---- k/v projections + earliest per-group staging ----------------
        for g in range(NKV):
            kproj(g)
            vproj(g)
            stage_cc(g)

        # ---- q projection (fills the gather window) ----------------------
        for m in range(NH):
            qproj(m)

        # ---- attention per head group ------------------------------------
        for i in range(NKV):
            # import gather i on the Pool queue right before its consumers
            for r in range(4):
                row = gths[i][r, :].rearrange("(p j) -> p j", p=128)
                nc.gpsimd.dma_start(kts[i][:, bass.ts(r, 512)], row[:, 0:512])
                nc.gpsimd.dma_start(vimp[i][:, bass.ts(r, 512)],
                                    row[:, 512:1024])
            for h in range(4 * i, 4 * i + 4):
                pvq = pp.tile([128, 512], f32, tag="acc4", bufs=2,
                              name="pvq")
                pend = []
                for pr in range(SJ // 2):
                    j0 = 2 * pr
                    sc = pp.tile([128, 1024], f32, tag="sc", bufs=2,
                                 name="sc")
                    nc.tensor.matmul(
                        sc[:, 0:512], kts[i][:, bass.ts(j0, 128)], qts[h][:],
                        start=True, stop=True,
                    )
                    nc.tensor.matmul(
                        sc[:, 512:1024], kts[i][:, bass.ts(j0 + 1, 128)],
                        qts[h][:], start=True, stop=True,
                    )
                    ex = at.tile([128, 1024], bf16, tag="ex", bufs=4,
                                 name="ex")
                    nc.scalar.activation(ex[:], sc[:], FP.Exp, scale=SCALE)
                    nc.tensor.matmul(
                        pvq[:], vimp[i][:, bass.ts(j0, 128)], ex[:, 0:512],
                        start=(pr == 0), stop=False,
                    )
                    nc.tensor.matmul(
                        pvq[:], vimp[i][:, bass.ts(j0 + 1, 128)],
                        ex[:, 512:1024],
                        start=False, stop=(pr == SJ // 2 - 1),
                    )
                    u = at.tile([128, 512], bf16, tag="uacc", bufs=6,
                                name="u")
                    nc.vector.tensor_add(u[:], ex[:, 0:512], ex[:, 512:1024])
                    pend.append(u)
                    while len(pend) >= 2 and pr % 2 == 1:
                        a = pend.pop(0)
                        b = pend.pop(0)
                        s2 = at.tile([128, 512], bf16, tag="uacc", bufs=6,
                                     name="s")
                        nc.vector.tensor_add(s2[:], a[:], b[:])
                        pend.append(s2)
                while len(pend) >= 2:
                    a = pend.pop(0)
                    b = pend.pop(0)
                    s2 = at.tile([128, 512], bf16, tag="uacc", bufs=6,
                                 name="s")
                    nc.vector.tensor_add(s2[:], a[:], b[:])
                    pend.append(s2)
                rsb = pp.tile([128, 512], f32, tag="mm", bufs=2, name="rsb")
                nc.tensor.matmul(rsb[:], ones[:], pend[0][:],
                                 start=True, stop=True)
                recipb = at.tile([128, 512], f32, tag="recipb", bufs=2,
                                 name="rc")
                with nc.allow_low_precision(reason="1/rowsum feeds bf16"):
                    nc.vector.reciprocal(recipb[:], rsb[:])
                # ctx8 = e4m3(ctx * s_c); ctxb = bf16 same; ctxr = ctxb-ctx8
                c8s = ctx8[:, h * 512:(h + 1) * 512]
                crs = ctxr[:, h * 512:(h + 1) * 512]
                with nc.allow_low_precision(reason="fp8 ctx + residual"):
                    nc.vector.scalar_tensor_tensor(
                        c8s, pvq[:], norm_sc, recipb[:],
                        op0=ALU.mult, op1=ALU.mult)
                    cb = at.tile([128, 512], bf16, tag="ctxb", bufs=2,
                                 name="cb")
                    nc.vector.scalar_tensor_tensor(
                        cb[:], pvq[:], norm_sc, recipb[:],
                        op0=ALU.mult, op1=ALU.mult)
                    nc.vector.scalar_tensor_tensor(
                        crs, c8s, -1.0, cb[:],
                        op0=ALU.mult, op1=ALU.add)

        # ---- o_proj tail: 3 fp8 DR streams, 4 live PSUM groups -----------
        ctx83 = ctx8[:].rearrange("p (h j) -> p h j", j=512)
        ctxr3 = ctxr[:].rearrange("p (h j) -> p h j", j=512)
        for n in range(4):
            wo8t = ot.tile([128, NH * 512], f8, tag="wo8", bufs=2,
                           name=f"wo8t{n}")
            wort = ot.tile([128, NH * 512], f8, tag="wor", bufs=2,
                           name=f"wort{n}")
            nc.sync.dma_start(wo8t[:], wo8P[:, bass.ts(n, NH * 512)])
            nc.sync.dma_start(wort[:], worP[:, bass.ts(n, NH * 512)])
            wo83 = wo8t[:].rearrange("p (h j) -> p h j", j=512)
            wor3 = wort[:].rearrange("p (h j) -> p h j", j=512)
            psos = [pp.tile([128, 512], f32, tag=("mm" if i < 2 else "acc4"),
                            bufs=2, name=f"pso{i}") for i in range(4)]
            idx = [0] * 4
            for tp in range(NH // 2):
                for xs3, ws3 in ((ctx83, wo83), (ctx83, wor3),
                                 (ctxr3, wo83)):
                    for sqt in range(4):
                        nc.tensor.matmul(
                            psos[sqt][:],
                            xs3[:, 2 * tp:2 * tp + 2,
                                sqt * 128:(sqt + 1) * 128],
                            ws3[:, 2 * tp:2 * tp + 2, :],
                            start=(idx[sqt] == 0),
                            stop=(idx[sqt] == 3 * (NH // 2) - 1),
                            perf_mode=DR,
                        )
                        idx[sqt] += 1
            for sqt in range(4):
                o_s = ot.tile([128, 512], f32, tag="osb", bufs=2, name="osb")
                nc.vector.scalar_tensor_tensor(
                    o_s[:], psos[sqt][:], o_sc, ones512[:],
                    op0=ALU.mult, op1=ALU.mult)
                nc.sync.dma_start(
                    outD[bass.ts(sqt, 128), bass.ts(n, 512)], o_s[:]
                )


def build(scales, reps=1):
    s_h, s_qk, s_wv, s_wo = scales
    norm_sc = float(S_C / (s_h * s_wv))
    o_sc = float(1.0 / (S_C * s_wo))
    nc = bacc.Bacc("TRN2", target_bir_lowering=False, debug=False,
                   num_devices=NC)
    t = {
        "h8P": nc.dram_tensor("h8P", [128, KT * 512], f8,
                              kind="ExternalInput").ap(),
        "hrP": nc.dram_tensor("hrP", [128, KT * 512], f8,
                              kind="ExternalInput").ap(),
        "wq8P": nc.dram_tensor("wq8P", [128, NH * KT * 128], f8,
                               kind="ExternalInput").ap(),
        "wqrP": nc.dram_tensor("wqrP", [128, NH * KT * 128], f8,
                               kind="ExternalInput").ap(),
        "wk8P": nc.dram_tensor("wk8P", [128, NKV * KT * 128], f8,
                               kind="ExternalInput").ap(),
        "wkrP": nc.dram_tensor("wkrP", [128, NKV * KT * 128], f8,
                               kind="ExternalInput").ap(),
        "wv8P": nc.dram_tensor("wv8P", [128, NKV * KT * 128], f8,
                               kind="ExternalInput").ap(),
        "wvrP": nc.dram_tensor("wvrP", [128, NKV * KT * 128], f8,
                               kind="ExternalInput").ap(),
        "wo8P": nc.dram_tensor("wo8P", [128, 4 * NH * 512], f8,
                               kind="ExternalInput").ap(),
        "worP": nc.dram_tensor("worP", [128, 4 * NH * 512], f8,
                               kind="ExternalInput").ap(),
        "ccss": nc.dram_tensor("ccss", [128, 1024], bf16,
                               kind="ExternalInput").ap(),
        "out": nc.dram_tensor("out", [SQ, H], f32, kind="ExternalOutput").ap(),
    }
    for g in range(NKV):
        t[f"stg{g}"] = nc.dram_tensor(f"stg{g}", [1, CHUNK], bf16,
                                      kind="Internal").ap()
        t[f"gth{g}"] = nc.dram_tensor(f"gth{g}", [4, CHUNK], bf16,
                                      kind="Internal").ap()
    with tile.TileContext(nc) as tc:
        for _ in range(reps):
            _body(nc, tc, t, norm_sc=norm_sc, o_sc=o_sc)
    nc.compile()
    return nc


_ROPE_PERM = np.concatenate(
    [h * HD + np.r_[np.arange(0, HD, 2), np.arange(1, HD, 2)]
     for h in range(NH)]
)
_ROPE_PERM_KV = _ROPE_PERM[: NKV * HD]


def _pow2_scale(x):
    return float(2.0 ** np.floor(np.log2(160.0 / np.abs(x).max())))


def _split8(x, s):
    xs = x * s
    x8 = xs.astype(F8NP)
    xr = (xs - x8.astype(np.float32)).astype(F8NP)
    return x8, xr


def _pack_w(w8, nh):
    # [p, m*KT*128 + k*128 + j] = wT[k*128+p, m*128+j]
    return np.ascontiguousarray(
        w8.reshape(KT, 128, nh, 128).transpose(1, 2, 0, 3).reshape(
            128, nh * KT * 128))


def prep_inputs(hidden_states, freqs_cos, freqs_sin, Wq, Wk, Wv, Wo):
    """Host-side fp8 residual split + layout prep -> (in_maps, scales)."""
    s_h = _pow2_scale(hidden_states)
    s_qk = min(_pow2_scale(Wq), _pow2_scale(Wk))
    s_wv = _pow2_scale(Wv)
    s_wo = _pow2_scale(Wo)

    wqT = np.ascontiguousarray(Wq.T[:, _ROPE_PERM])
    wkT = np.ascontiguousarray(Wk.T[:, _ROPE_PERM_KV])
    wvT = np.ascontiguousarray(Wv.T)
    woT = np.ascontiguousarray(Wo.T)

    wq8, wqr = _split8(wqT, s_qk)
    wk8, wkr = _split8(wkT, s_qk)
    wv8, wvr = _split8(wvT, s_wv)
    wo8, wor = _split8(woT, s_wo)

    wq8P, wqrP = _pack_w(wq8, NH), _pack_w(wqr, NH)
    wk8P, wkrP = _pack_w(wk8, NKV), _pack_w(wkr, NKV)
    wv8P, wvrP = _pack_w(wv8, NKV), _pack_w(wvr, NKV)

    def pack_wo(w):
        return np.ascontiguousarray(
            w.reshape(NH, 128, 4, 512).transpose(1, 2, 0, 3).reshape(
                128, 4 * NH * 512))

    wo8P, worP = pack_wo(wo8), pack_wo(wor)

    cosT = freqs_cos.T  # [64, S]
    sinT = freqs_sin.T
    rope_sc = 1.0 / (s_h * s_qk)
    cc_full = np.concatenate([cosT, cosT], 0) * rope_sc  # [128, S]
    ss_full = np.concatenate([sinT, sinT], 0) * rope_sc

    in_maps = []
    for c in range(NC):
        b, chunk = divmod(c, 4)
        sq0 = chunk * SQ
        hTc = np.ascontiguousarray(hidden_states[b].T[:, sq0:sq0 + SQ])
        h8, hr = _split8(hTc, s_h)

        def pack_h(hh):
            return np.ascontiguousarray(
                hh.reshape(KT, 128, 512).transpose(1, 0, 2).reshape(
                    128, KT * 512))

        ccss = np.ascontiguousarray(np.concatenate(
            [cc_full[:, sq0:sq0 + SQ], ss_full[:, sq0:sq0 + SQ]],
            axis=1)).astype(BF16)
        in_maps.append(
            {"h8P": pack_h(h8), "hrP": pack_h(hr),
             "wq8P": wq8P, "wqrP": wqrP, "wk8P": wk8P, "wkrP": wkrP,
             "wv8P": wv8P, "wvrP": wvrP, "wo8P": wo8P, "worP": worP,
             "ccss": ccss}
        )
    return in_maps, (s_h, s_qk, s_wv, s_wo)


_CACHE = {}


def _get_nc(scales=None, reps=1):
    if scales is None:
        scales = _CACHE["last_scales"]
    key = (scales, reps)
    if key not in _CACHE:
        _CACHE[key] = build(scales, reps)
    _CACHE["last_scales"] = scales
    return _CACHE[key]


def kernel(hidden_states, freqs_cos, freqs_sin, Wq, Wk, Wv, Wo):
    in_maps, scales = prep_inputs(
        np.asarray(hidden_states, np.float32),
        np.asarray(freqs_cos, np.float32),
        np.asarray(freqs_sin, np.float32),
        np.asarray(Wq, np.float32),
        np.asarray(Wk, np.float32),
        np.asarray(Wv, np.float32),
        np.asarray(Wo, np.float32),
    )
    nc = _get_nc(scales, int(os.environ.get("KERNEL_REPS", "1")))
    res = run_bass_kernel_spmd(nc, in_maps, core_ids=list(range(NC)))
    out = np.empty((B, S, H), np.float32)
    for c in range(NC):
        b, chunk = divmod(c, 4)
        out[b, chunk * SQ: (chunk + 1) * SQ, :] = res.results[c]["out"]
    return out
